# revision 3
# baseline (speedup 1.0000x reference)
"""Chamfer distance loss on 8 TRN2 NeuronCores.

Strategy (data-parallel over batch, 4 batches per core):
  - Host gathers the 2048-point subsets p1 = points1[:, idx1], p2 = points2[:, idx2].
  - Squared pairwise distances are computed on TensorE as a K=24 augmented
    matmul: D[s,t] = n1_s + n2_t - 2*p1_s.p2_t, with every f32 quantity split
    into 3 bf16 components (hi/mid/lo) so all products are exact in the PE
    array's bf16*bf16->f32 datapath. Matrix abs error ~8e-7.
  - VectorE reduce_min over each 128x2048 PSUM row block gives per-point min
    squared distances for both chamfer directions.
  - Host does the final sqrt / means over the 8 cores' outputs (65K values).
"""

import numpy as np
import ml_dtypes

import concourse.bass as bass
from concourse import bacc
import concourse.tile as tile
from concourse import mybir
from concourse.bass_utils import run_bass_kernel_spmd

BF16 = ml_dtypes.bfloat16

B = 32              # global batch
S = 2048            # sampled points per cloud
N_CORES = 8
B_LOC = B // N_CORES  # batches per core
N_CHUNKS = S // 128   # s-chunks per (batch, direction)
N_JT = S // 512       # 512-wide column tiles per row block


def _split3(x):
    """Split float64 array into 3 bf16 components (as float32 exact values)."""
    h = x.astype(BF16).astype(np.float64)
    m = (x - h).astype(BF16).astype(np.float64)
    l = (x - h - m).astype(BF16).astype(np.float64)
    return h, m, l


def _build_aug(a, b):
    """a, b: (S, 3) float64 point sets. Returns A, B: (24, S) float32 arrays of
    bf16-representable values such that A[:, s] . B[:, t] == |a_s - b_t|^2 up
    to ~1e-6 abs error."""
    ah, am, al = _split3(a)
    bh, bm, bl = _split3(b)
    n1h, n1m, n1l = _split3((a * a).sum(1))
    n2h, n2m, n2l = _split3((b * b).sum(1))
    A = np.zeros((24, a.shape[0]))
    Bm = np.zeros((24, b.shape[0]))
    pairs = [(ah, bh), (ah, bm), (am, bh), (ah, bl), (al, bh), (am, bm)]
    for k, (x, y) in enumerate(pairs):
        A[3 * k:3 * k + 3] = (-2.0 * x).T
        Bm[3 * k:3 * k + 3] = y.T
    A[18], A[19], A[20] = n1h, n1m, n1l
    Bm[18:21] = 1.0
    A[21:24] = 1.0
    Bm[21], Bm[22], Bm[23] = n2h, n2m, n2l
    return A.astype(BF16), Bm.astype(BF16)


def _build_nc():
    """One NeuronCore program (SPMD across 8 cores).

    Inputs : a1, b2  (24, B_LOC*S) bf16 — augmented encodings, batches
             side-by-side along the free dim.
    Output : out (128, B_LOC*2*N_CHUNKS) f32 — per-point min squared
             distances; column = (b_local*2 + dir)*N_CHUNKS + chunk,
             partition = point index within chunk.
    """
    nc = bacc.Bacc()
    a1_d = nc.declare_dram_parameter("a1", [24, B_LOC * S], mybir.dt.bfloat16, isOutput=False)
    b2_d = nc.declare_dram_parameter("b2", [24, B_LOC * S], mybir.dt.bfloat16, isOutput=False)
    out_d = nc.declare_dram_parameter("out", [128, B_LOC * 2 * N_CHUNKS], mybir.dt.float32, isOutput=True)

    with tile.TileContext(nc) as tc:
        with (
            tc.tile_pool(name="inp", bufs=1) as inp,
            tc.tile_pool(name="mins", bufs=1) as minp,
            tc.tile_pool(name="psum", bufs=2, space="PSUM") as psp,
        ):
            a1 = inp.tile([24, B_LOC * S], mybir.dt.bfloat16)
            b2 = inp.tile([24, B_LOC * S], mybir.dt.bfloat16)
            nc.sync.dma_start(a1[:], a1_d[:])
            nc.sync.dma_start(b2[:], b2_d[:])

            mins = minp.tile([128, B_LOC * 2 * N_CHUNKS], mybir.dt.float32)

            for b in range(B_LOC):
                for dr in range(2):
                    lhs_src, rhs_src = (a1, b2) if dr == 0 else (b2, a1)
                    for c in range(N_CHUNKS):
                        ps = psp.tile([128, S], mybir.dt.float32)
                        lhsT = lhs_src[0:24, b * S + c * 128: b * S + (c + 1) * 128]
                        for j in range(N_JT):
                            rhs = rhs_src[0:24, b * S + j * 512: b * S + (j + 1) * 512]
                            nc.tensor.matmul(ps[:, bass.ts(j, 512)], lhsT, rhs,
                                             start=True, stop=True)
                        col = (b * 2 + dr) * N_CHUNKS + c
                        nc.vector.tensor_reduce(
                            out=mins[:, col:col + 1], in_=ps[:],
                            axis=mybir.AxisListType.X, op=mybir.AluOpType.min)

            nc.sync.dma_start(out_d[:], mins[:])
    return nc


_NC_CACHE = None


def _get_nc():
    global _NC_CACHE
    if _NC_CACHE is None:
        nc = _build_nc()
        if not nc.is_finalized():
            nc.finalize()
        _NC_CACHE = nc
    return _NC_CACHE


def _prepare_in_maps(points1, points2, idx1, idx2):
    g1 = np.asarray(points1)[:, np.asarray(idx1)].astype(np.float64)  # (B,S,3)
    g2 = np.asarray(points2)[:, np.asarray(idx2)].astype(np.float64)
    in_maps = []
    for core in range(N_CORES):
        A = np.empty((24, B_LOC * S), dtype=BF16)
        Bm = np.empty((24, B_LOC * S), dtype=BF16)
        for bl in range(B_LOC):
            bg = core * B_LOC + bl
            Ab, Bb = _build_aug(g1[bg], g2[bg])
            A[:, bl * S:(bl + 1) * S] = Ab
            Bm[:, bl * S:(bl + 1) * S] = Bb
        in_maps.append({"a1": A, "b2": Bm})
    return in_maps


def _reduce_outputs(results):
    """results: list of 8 dicts with 'out' (128, B_LOC*2*N_CHUNKS) f32."""
    total = 0.0
    for core in range(N_CORES):
        out = np.asarray(results[core]["out"], dtype=np.float64)
        # column = (b*2 + dir)*N_CHUNKS + chunk ; partition = within-chunk idx
        d2 = out.reshape(128, B_LOC * 2, N_CHUNKS).transpose(1, 2, 0)  # (bd, chunk, 128)
        d2 = d2.reshape(B_LOC * 2, S)
        dist = np.sqrt(np.maximum(d2, 0.0))
        total += dist.mean(axis=1).sum()  # sum over batches and directions
    return np.float32(total / B)


def _run(inputs, trace=False):
    nc = _get_nc()
    in_maps = _prepare_in_maps(inputs["points1"], inputs["points2"],
                               inputs["idx1"], inputs["idx2"])
    res = run_bass_kernel_spmd(nc, in_maps, core_ids=list(range(N_CORES)),
                               trace=trace)
    loss = _reduce_outputs(res.results)
    return loss, res


def kernel(**inputs):
    loss, _ = _run(inputs, trace=False)
    return loss


# revision 20
# speedup vs baseline: 3335.1061x; 3335.1061x over previous
"""Chamfer distance loss on 8 TRN2 NeuronCores.

Strategy (data-parallel over batch, 4 batches per core):
  - Host gathers the 2048-point subsets p1 = points1[:, idx1], p2 = points2[:, idx2].
  - Squared pairwise distances are computed on TensorE as a K=24 augmented
    matmul: D[s,t] = n1_s + n2_t - 2*p1_s.p2_t, with every f32 quantity split
    into 3 bf16 components (hi/mid/lo) so all products are exact in the PE
    array's bf16*bf16->f32 datapath. Matrix abs error ~1e-6.
  - Exact windowed NN: both point sets are sorted along x on the host. For
    each 128-query chunk a provably sufficient target window is derived from
    cheap host-side NN upper bounds (u_s = best distance among 2K rank
    neighbors; the true NN must satisfy |x_t - x_s| <= u_s). The device only
    computes the distance block for that window (~3x fewer elements).
  - SPMD-safe slotting: each core sorts its 128 (batch, dir, chunk) units by
    window width; slot k of the shared program uses width SCHED[k] >= every
    core's k-th widest unit. Slot operands are placed in PE row-group k%4
    (partitions 32g..32g+23) so consecutive matmuls run concurrently.
  - Drain is split across both PSUM-capable engines: VectorE reduce_min
    directly from PSUM, and ScalarE copy->SBUF(bf16) followed by a fused
    VectorE tensor_tensor_reduce(min) for the rest.
  - Host does the final sqrt / means over the 8 cores' outputs (65K values).
"""

import os
import numpy as np
import ml_dtypes

import concourse.bass as bass
from concourse import bacc
import concourse.tile as tile
from concourse import mybir
from concourse.bass_utils import run_bass_kernel_spmd

BF16 = ml_dtypes.bfloat16

B = 32              # global batch
S = 2048            # sampled points per cloud
N_CORES = 8
B_LOC = B // N_CORES  # batches per core
N_CHUNKS = S // 128   # query chunks per (batch, direction)
N_UNITS = B_LOC * 2 * N_CHUNKS  # 128 slots per core
KC = 16             # rank-neighbor candidates per side for the NN upper bound

# Slot width schedule computed from the reference inputs (max over cores of
# the k-th widest window, padded to 256). Recomputed at runtime if the actual
# inputs need wider windows (forces a recompile but stays correct).
SCHED_DEFAULT = None  # filled lazily from data; kept for documentation


# ---------------------------------------------------------------- host math

def _split3(x):
    h = x.astype(BF16).astype(np.float64)
    m = (x - h).astype(BF16).astype(np.float64)
    l = (x - h - m).astype(BF16).astype(np.float64)
    return h, m, l


def _build_aug(a, b):
    """a, b: (S, 3) float64 (p1-side, p2-side) point sets. Returns A, B:
    (24, S) bf16 with A[:, s] . B[:, t] == |a_s - b_t|^2 up to ~1e-6."""
    ah, am, al = _split3(a)
    bh, bm, bl = _split3(b)
    n1h, n1m, n1l = _split3((a * a).sum(1))
    n2h, n2m, n2l = _split3((b * b).sum(1))
    A = np.zeros((24, a.shape[0]))
    Bm = np.zeros((24, b.shape[0]))
    pairs = [(ah, bh), (ah, bm), (am, bh), (ah, bl), (al, bh), (am, bm)]
    for k, (x, y) in enumerate(pairs):
        A[3 * k:3 * k + 3] = (-2.0 * x).T
        Bm[3 * k:3 * k + 3] = y.T
    A[18], A[19], A[20] = n1h, n1m, n1l
    Bm[18:21] = 1.0
    A[21:24] = 1.0
    Bm[21], Bm[22], Bm[23] = n2h, n2m, n2l
    return A.astype(BF16), Bm.astype(BF16)


def _morton_key(p):
    q = np.clip(((p + 4.0) / 8.0 * 1024).astype(np.int64), 0, 1023)

    def spread(x):
        x = (x | (x << 16)) & 0x030000FF
        x = (x | (x << 8)) & 0x0300F00F
        x = (x | (x << 4)) & 0x030C30C3
        x = (x | (x << 2)) & 0x09249249
        return x

    return spread(q[:, 0]) | (spread(q[:, 1]) << 1) | (spread(q[:, 2]) << 2)


def _unit_windows(q, t):
    """q, t: (S, 3) float64, both sorted by x. For each 128-query chunk,
    return (lo, width) of a target-rank window guaranteed to contain every
    query's true nearest neighbor. The upper bound u_s is the best distance
    among 2*KC x-rank neighbors and 2*KC Morton-order neighbors."""
    pos = np.searchsorted(t[:, 0], q[:, 0]).clip(0, S - 1)
    idx = (pos[:, None] + np.arange(-KC, KC)[None, :]).clip(0, S - 1)
    u2 = ((q[:, None, :] - t[idx]) ** 2).sum(-1).min(1)
    mq, mt = _morton_key(q), _morton_key(t)
    to = np.argsort(mt, kind="stable")
    ts = t[to]
    posm = np.searchsorted(mt[to], mq).clip(0, S - 1)
    idxm = (posm[:, None] + np.arange(-KC, KC)[None, :]).clip(0, S - 1)
    u2 = np.minimum(u2, ((q[:, None, :] - ts[idxm]) ** 2).sum(-1).min(1))
    u = np.sqrt(u2) * (1 + 1e-9)
    lo_x = q[:, 0] - u
    hi_x = q[:, 0] + u
    wins = []
    for ch in range(N_CHUNKS):
        sl = slice(ch * 128, ch * 128 + 128)
        lo = int(np.searchsorted(t[:, 0], lo_x[sl].min(), side="left"))
        hi = int(np.searchsorted(t[:, 0], hi_x[sl].max(), side="right"))
        wins.append((lo, hi - lo))
    return wins


def _prepare(points1, points2, idx1, idx2):
    """Returns (cores, widths) where cores[i] holds per-core staging data and
    widths is an (N_CORES, N_UNITS) array of desc-sorted window widths."""
    g1 = np.asarray(points1)[:, np.asarray(idx1)].astype(np.float64)
    g2 = np.asarray(points2)[:, np.asarray(idx2)].astype(np.float64)
    cores = []
    widths = np.zeros((N_CORES, N_UNITS), dtype=np.int64)
    for core in range(N_CORES):
        augs = []   # per bl: (A1s, B2s)
        units = []  # (w, lo, bl, dr, ch)
        for bl in range(B_LOC):
            b = core * B_LOC + bl
            a = g1[b][np.argsort(g1[b][:, 0], kind="stable")]
            c = g2[b][np.argsort(g2[b][:, 0], kind="stable")]
            augs.append(_build_aug(a, c))
            for dr, (q, t) in enumerate(((a, c), (c, a))):
                for ch, (lo, w) in enumerate(_unit_windows(q, t)):
                    units.append((w, lo, bl, dr, ch))
        units.sort(key=lambda u: -u[0])
        widths[core] = [u[0] for u in units]
        cores.append({"augs": augs, "units": units})
    return cores, widths


def _schedule(widths):
    need = widths.max(axis=0)
    sched = (np.ceil(np.maximum(need, 256) / 256).astype(np.int64) * 256).clip(max=S)
    return [int(w) for w in sched]


def _path_costs(W, kind):
    """(scalar_ns, vector_ns) estimates per drain path."""
    if kind == "D":      # per-bank DVE reduces from PSUM
        nb = (W + 511) // 512
        return 0.0, (W + nb * 120) / 0.96
    if kind == "A":      # ScalarE copy -> bf16 SBUF, DVE fold(s)+reduce
        if W >= 1280:
            v = (W / 4 + 151) + (W / 8 + 151) + (W / 4 + 58)
        else:
            v = (W / 4 + 151) + (W / 2 + 58)
        return (W + 352) / 1.2, v / 0.96
    if kind == "M":      # ScalarE negated copy -> bf16 SBUF, DVE Max8
        return (W + 352) / 1.2, (W / 2 + 60) / 0.96
    raise ValueError(kind)


def _plan_paths(sched):
    """Greedy split of slots between drain paths, balancing engine times."""
    forced = os.environ.get("CHAMFER_PATHS", "")  # "", "D", "A", "M", "DA", "DM"
    if forced in ("D", "A", "M"):
        return [forced] * len(sched)
    kinds = ("D", "A", "M") if not forced else tuple(forced)
    if not os.environ.get("CHAMFER_USE_M"):
        kinds = tuple(k for k in kinds if k != "M")
    tS = tV = 0.0
    paths = []
    for W in sched:
        best, bs, bv = None, None, None
        for kind in kinds:
            cs, cv = _path_costs(W, kind)
            m = max(tS + cs, tV + cv)
            if best is None or m < best:
                best, bs, bv, bk = m, tS + cs, tV + cv, kind
        tS, tV = bs, bv
        paths.append(bk)
    return paths


# ------------------------------------------------------------- device build

N_GROUPS = int(os.environ.get("CHAMFER_GROUPS", "4"))


def _slot_layout(sched):
    """Group/offset layout: slot k lives in PE row-group k%N_GROUPS at column
    offset off[k] of the shared window buffer."""
    off = [0] * len(sched)
    gsum = [0] * N_GROUPS
    for k, W in enumerate(sched):
        g = k % N_GROUPS
        off[k] = gsum[g]
        gsum[g] += W
    return off, max(gsum)


def _build_nc_v2(sched, paths, reps=1):
    off, gc = _slot_layout(sched)
    nc = bacc.Bacc()
    tq_d = nc.declare_dram_parameter("tq", [128, ((N_UNITS + N_GROUPS - 1) // N_GROUPS) * 128], mybir.dt.bfloat16, isOutput=False)
    wb_d = nc.declare_dram_parameter("wb", [128, gc], mybir.dt.bfloat16, isOutput=False)
    out_d = nc.declare_dram_parameter("out", [128, 4 * N_UNITS], mybir.dt.float32, isOutput=True)
    use_m = any(p == "M" for p in paths)
    out8_d = nc.declare_dram_parameter(
        "out8", [128, 8 * N_UNITS] if use_m else [1, 8], mybir.dt.float32, isOutput=True)

    with tile.TileContext(nc) as tc:
        with (
            tc.tile_pool(name="inp", bufs=1) as inp,
            tc.tile_pool(name="sb", bufs=3) as sbp,
            tc.tile_pool(name="aux", bufs=1) as aux,
            tc.tile_pool(name="psum", bufs=2, space="PSUM") as psp,
        ):
            tq = inp.tile([128, ((N_UNITS + N_GROUPS - 1) // N_GROUPS) * 128], mybir.dt.bfloat16)
            wb = inp.tile([128, gc], mybir.dt.bfloat16)
            nc.sync.dma_start(tq[:], tq_d[:])
            half = (gc // 2) & ~255
            nc.sync.dma_start(wb[:, :half], wb_d[:, :half])
            nc.sync.dma_start(wb[:, half:], wb_d[:, half:])

            mins = aux.tile([128, 4 * N_UNITS], mybir.dt.float32)
            nc.vector.memset(mins[:], 3.0e38)
            junk = aux.tile([128, S], mybir.dt.bfloat16)
            junkf = aux.tile([128, 8], mybir.dt.float32)
            mins8 = aux.tile([128, 8 * N_UNITS], mybir.dt.float32, name="mins8") if use_m else None

            tiny_reduce = bool(os.environ.get("CHAMFER_TINY_REDUCE"))
            no_mm = bool(os.environ.get("CHAMFER_NO_MM"))

            def body(_i=None):
                for k, W in enumerate(sched):
                    g, kg = k % N_GROUPS, k // N_GROUPS
                    p0 = 32 * g
                    ps = psp.tile([128, S], mybir.dt.float32)
                    lhsT = tq[p0:p0 + 24, kg * 128:(kg + 1) * 128]
                    if no_mm:
                        nc.vector.memset(ps[:, 0:W], 0.0)
                    else:
                        for j in range(0, W, 512):
                            n = min(512, W - j)
                            rhs = wb[p0:p0 + 24, off[k] + j: off[k] + j + n]
                            nc.tensor.matmul(ps[:, j:j + n], lhsT, rhs,
                                             start=True, stop=True, tile_position=(p0, 0))
                    if tiny_reduce:
                        nc.vector.tensor_reduce(
                            out=mins[:, 4 * k:4 * k + 1], in_=ps[:, :128],
                            axis=mybir.AxisListType.X, op=mybir.AluOpType.min)
                    elif paths[k] == "D":
                        # one reduce per PSUM bank (multi-bank DVE PSUM APs hit a
                        # catastrophic HW slow path); host mins the partials
                        for jb in range(0, W, 512):
                            hi2 = min(jb + 512, W)
                            nc.vector.tensor_reduce(
                                out=mins[:, 4 * k + jb // 512:4 * k + jb // 512 + 1],
                                in_=ps[:, jb:hi2],
                                axis=mybir.AxisListType.X, op=mybir.AluOpType.min)
                    elif paths[k] == "A":
                        sb = sbp.tile([128, S], mybir.dt.bfloat16)
                        if os.environ.get("CHAMFER_SPLIT_COPY"):
                            for jb in range(0, W, 512):
                                hi2 = min(jb + 512, W)
                                nc.scalar.copy(out=sb[:, jb:hi2], in_=ps[:, jb:hi2])
                        else:
                            nc.scalar.copy(out=sb[:, :W], in_=ps[:, :W])
                        h = W // 2
                        nc.vector.tensor_tensor(
                            out=junk[:, :h], in0=sb[:, :h], in1=sb[:, h:W],
                            op=mybir.AluOpType.min)
                        if W >= 1280:
                            q = h // 2
                            nc.vector.tensor_tensor(
                                out=junk[:, h:h + q], in0=junk[:, :q], in1=junk[:, q:h],
                                op=mybir.AluOpType.min)
                            nc.vector.tensor_reduce(
                                out=mins[:, 4 * k:4 * k + 1], in_=junk[:, h:h + q],
                                axis=mybir.AxisListType.X, op=mybir.AluOpType.min)
                        else:
                            nc.vector.tensor_reduce(
                                out=mins[:, 4 * k:4 * k + 1], in_=junk[:, :h],
                                axis=mybir.AxisListType.X, op=mybir.AluOpType.min)
                    else:  # "M": negated copy + Max8 (mins = -out8[:, 8k])
                        sb = sbp.tile([128, S], mybir.dt.bfloat16)
                        if os.environ.get("CHAMFER_SPLIT_COPY"):
                            for jb in range(0, W, 512):
                                hi2 = min(jb + 512, W)
                                nc.scalar.mul(out=sb[:, jb:hi2], in_=ps[:, jb:hi2], mul=-1.0)
                        else:
                            nc.scalar.mul(out=sb[:, :W], in_=ps[:, :W], mul=-1.0)
                        nc.vector.max(out=mins8[:, 8 * k:8 * k + 8], in_=sb[:, :W])

            if reps > 1 and os.environ.get("CHAMFER_UNROLL"):
                for _ in range(reps):
                    body()
            elif reps > 1:
                with tc.For_i(0, reps, 1):
                    body()
            else:
                body()

            nc.sync.dma_start(out_d[:], mins[:])
            if use_m:
                nc.sync.dma_start(out8_d[:], mins8[:])
            else:
                zz = aux.tile([1, 8], mybir.dt.float32)
                nc.vector.memset(zz[:], 0.0)
                nc.sync.dma_start(out8_d[:], zz[:])
    if not nc.is_finalized():
        nc.finalize()
    return nc


_NC_CACHE = {}


def _get_nc_v2(sched, paths, reps=1):
    key = (tuple(sched), tuple(paths), reps)
    if key not in _NC_CACHE:
        _NC_CACHE[key] = _build_nc_v2(sched, paths, reps)
    return _NC_CACHE[key]


def _make_in_maps(cores, sched):
    off, gc = _slot_layout(sched)
    in_maps = []
    for core in range(N_CORES):
        tq = np.zeros((128, ((N_UNITS + N_GROUPS - 1) // N_GROUPS) * 128), dtype=BF16)
        wb = np.zeros((128, gc), dtype=BF16)
        meta = []
        for k, (w, lo, bl, dr, ch) in enumerate(cores[core]["units"]):
            W = sched[k]
            g, kg = k % N_GROUPS, k // N_GROUPS
            p0 = 32 * g
            A1s, B2s = cores[core]["augs"][bl]
            qsrc, tsrc = (A1s, B2s) if dr == 0 else (B2s, A1s)
            tq[p0:p0 + 24, kg * 128:(kg + 1) * 128] = qsrc[:, ch * 128:(ch + 1) * 128]
            lo2 = min(max(lo - (W - w) // 2, 0), S - W)
            wb[p0:p0 + 24, off[k]:off[k] + W] = tsrc[:, lo2:lo2 + W]
            meta.append((bl, dr))
        in_maps.append({"tq": tq, "wb": wb})
        cores[core]["meta"] = meta
    return in_maps


def _reduce_outputs_v2(results, cores, paths):
    total = 0.0
    m_slots = [k for k, p in enumerate(paths) if p == "M"]
    for core in range(N_CORES):
        raw = np.asarray(results[core]["out"], dtype=np.float64)  # (128, 4*N_UNITS)
        out = raw.reshape(128, N_UNITS, 4).min(axis=2)
        if m_slots:
            out8 = np.asarray(results[core]["out8"], dtype=np.float64)
            for k in m_slots:
                out[:, k] = -out8[:, 8 * k]
        dist = np.sqrt(np.maximum(out, 0.0))
        total += dist.sum() / S
    return np.float32(total / B)


def _run(inputs, trace=False, timers=None, reps=None):
    import time as _t
    if reps is None:
        reps = int(os.environ.get("CHAMFER_REPS", "1"))
    t0 = _t.time()
    cores, widths = _prepare(inputs["points1"], inputs["points2"],
                             inputs["idx1"], inputs["idx2"])
    sched = _schedule(widths)
    paths = _plan_paths(sched)
    nc = _get_nc_v2(sched, paths, reps)
    in_maps = _make_in_maps(cores, sched)
    t1 = _t.time()
    res = run_bass_kernel_spmd(nc, in_maps, core_ids=list(range(N_CORES)),
                               trace=trace)
    t2 = _t.time()
    loss = _reduce_outputs_v2(res.results, cores, paths)
    if timers is not None:
        timers["prepare_s"] = t1 - t0
        timers["run_s"] = t2 - t1
    return loss, res


def kernel(**inputs):
    loss, _ = _run(inputs, trace=False)
    return loss
